# revision 8
# baseline (speedup 1.0000x reference)
"""Trainium2 Bass kernel for masked pairwise-sigmoid GNN message passing.

Reference computation (per graph g with nodes i,j in [0,nv)):
    c = z @ Wc.T + bc ; y = z @ Wy.T + by          # [G, nv, H]
    s[g,i,j,:] = sigmoid(c[g,i,:] + y[g,j,:] + (m_i + m_j)*L - 2L)
    out[g,i,:] = sum_j s[g,i,j,:] / sum_j m[g,j]

Key exact identity: with m in {0,1}, any pair with m_i==0 or m_j==0 has
mask term <= -1e10, so sigmoid underflows to exactly 0 in fp32.  Hence
only "active" nodes (m==1) contribute, and for active pairs the mask
term is exactly 0.  The host gathers active nodes per graph, the device
computes the dense active x active interaction, and the host scatters
rows back (inactive rows are exactly 0).

Sharding: graphs are sorted by active count and dealt round-robin to the
8 cores in 4 "slots"; slot s is padded to a single global size P_s so
one SPMD program serves all cores.  Padding columns get a -1e5 additive
mask (sigmoid -> 0) and padding rows are discarded on scatter.

Device layout keeps the hidden dim on partitions throughout; the output
is returned transposed ([H, NTOT]) and the host untransposes during the
scatter (pure data movement).  All arithmetic including the 1/denom
scale happens on device.
"""

import numpy as np

import concourse.bass as bass
import concourse.mybir as mybir
import concourse.tile as tile
from concourse import bacc
from concourse.bass_utils import run_bass_kernel_spmd

F32 = mybir.dt.float32
F32R = mybir.dt.float32r
N_CORES = 8
PAD_NEG = -1.0e5  # additive mask for padding columns; sigmoid(-1e5) == 0

# test.py reads this for profiling info after a traced run
_last_results = None


def _build_program(P_list, H, gp_tt_obs=(1,)):
    """One-core program; SPMD-replicated over 8 cores with different data.

    gp_tt_obs: which h-blocks' pairwise adds run on GpSimd instead of DVE
    (DVE is the bottleneck; GpSimd is otherwise idle).
    """
    NTOT = sum(P_list)
    KB = H // 128  # contraction blocks
    OB = H // 128  # output h blocks
    assert H % 128 == 0

    nc = bacc.Bacc(None, target_bir_lowering=False)

    zT = nc.dram_tensor("zT", [H, NTOT], F32, kind="ExternalInput")
    wcT = nc.dram_tensor("wcT", [H, H], F32, kind="ExternalInput")
    wyT = nc.dram_tensor("wyT", [H, H], F32, kind="ExternalInput")
    bcT = nc.dram_tensor("bcT", [128, OB], F32, kind="ExternalInput")
    byT = nc.dram_tensor("byT", [128, OB], F32, kind="ExternalInput")
    madd = nc.dram_tensor("madd", [1, NTOT], F32, kind="ExternalInput")
    recipN = nc.dram_tensor("recipN", [1, NTOT], F32, kind="ExternalInput")
    out = nc.dram_tensor("out", [H, NTOT], F32, kind="ExternalOutput")

    AT = mybir.ActivationFunctionType
    OP = mybir.AluOpType

    with tile.TileContext(nc) as tc:
        with (
            tc.tile_pool(name="singles", bufs=1) as singles,
            tc.tile_pool(name="work", bufs=2) as work,
            tc.tile_pool(name="outp", bufs=2) as outp,
            tc.tile_pool(name="psum", bufs=2, space="PSUM") as psum,
        ):
            # ---- load inputs; spread big loads over distinct engine queues
            z_sb = []
            for kb in range(KB):
                t = singles.tile([128, NTOT], F32, tag=f"z{kb}", name=f"z{kb}")
                nc.gpsimd.dma_start(out=t[:], in_=zT[kb * 128:(kb + 1) * 128, :])
                z_sb.append(t)
            w_sb = {}
            w_engines = {"c": nc.sync, "y": nc.scalar}
            for wname, dram in (("c", wcT), ("y", wyT)):
                for kb in range(KB):
                    t = singles.tile(
                        [128, H], F32, tag=f"w{wname}{kb}", name=f"w{wname}{kb}"
                    )
                    # split the [128, H] load into per-o-block halves so the
                    # first projection's weights land sooner
                    eng = w_engines[wname]
                    for ob in range(OB):
                        eng.dma_start(
                            out=t[:, ob * 128:(ob + 1) * 128],
                            in_=dram[kb * 128:(kb + 1) * 128,
                                     ob * 128:(ob + 1) * 128],
                        )
                    w_sb[wname, kb] = t
            bc_sb = singles.tile([128, OB], F32, tag="bc", name="bc_sb")
            nc.gpsimd.dma_start(out=bc_sb[:], in_=bcT[:])
            by_sb = singles.tile([128, OB], F32, tag="by", name="by_sb")
            nc.gpsimd.dma_start(out=by_sb[:], in_=byT[:])
            # one-row loads, broadcast to 128 partitions on-chip
            madd_row = singles.tile([1, NTOT], F32, tag="maddr", name="madd_row")
            nc.gpsimd.dma_start(out=madd_row[:], in_=madd[:])
            recip_row = singles.tile([1, NTOT], F32, tag="recr", name="recip_row")
            nc.gpsimd.dma_start(out=recip_row[:], in_=recipN[:])
            madd_rep = singles.tile([128, NTOT], F32, tag="madd", name="madd_rep")
            nc.gpsimd.partition_broadcast(madd_rep[:], madd_row[:])
            recip_rep = singles.tile([128, NTOT], F32, tag="recip", name="recip_rep")
            nc.gpsimd.partition_broadcast(recip_rep[:], recip_row[:])

            # ---- projections: C'[o, n] = (Wc @ z^T)[o, n] + bc[o] (+madd for y)
            ct_sb = [
                singles.tile([128, NTOT], F32, tag=f"ct{ob}", name=f"ct{ob}")
                for ob in range(OB)
            ]
            yt_sb = [
                singles.tile([128, NTOT], F32, tag=f"yt{ob}", name=f"yt{ob}")
                for ob in range(OB)
            ]
            # order (c,ob0), (y,ob0), (c,ob1), (y,ob1): the first slot's
            # pairwise add needs ob0 of both projections as early as possible
            proj_jobs = []
            for ob in range(OB):
                proj_jobs.append(("c", ob, ct_sb, bc_sb, False))
                proj_jobs.append(("y", ob, yt_sb, by_sb, True))
            for wname, ob, dest, bias_sb, with_madd in proj_jobs:
                ps = psum.tile([128, NTOT], F32, name="ps")
                for kb in range(KB):
                    nc.tensor.matmul(
                        ps[:],
                        lhsT=w_sb[wname, kb][:, ob * 128:(ob + 1) * 128],
                        rhs=z_sb[kb][:],
                        start=(kb == 0),
                        stop=(kb == KB - 1),
                    )
                if with_madd:
                    nc.vector.scalar_tensor_tensor(
                        out=dest[ob][:],
                        in0=ps[:],
                        scalar=bias_sb[:, ob:ob + 1],
                        in1=madd_rep[:],
                        op0=OP.add,
                        op1=OP.add,
                    )
                else:
                    nc.vector.tensor_scalar_add(
                        out=dest[ob][:], in0=ps[:], scalar1=bias_sb[:, ob:ob + 1]
                    )

            # ---- main loop; per-slot scale + store
            out_sb = [
                singles.tile([128, NTOT], F32, tag=f"o{ob}", name=f"osb{ob}")
                for ob in range(OB)
            ]
            out_dma_engines = [nc.sync, nc.scalar]
            col = 0
            for si, P in enumerate(P_list):
                for ob in range(OB):
                    cpart = ct_sb[ob][:, col:col + P]  # [128, P] (i)
                    ypart = yt_sb[ob][:, col:col + P]  # [128, P] (j)
                    # in0[p, i, j] = c'[p, i]; in1[p, i, j] = y'[p, j]
                    in0 = bass.AP(
                        tensor=cpart.tensor,
                        offset=cpart.offset,
                        ap=[list(cpart.ap[0]), list(cpart.ap[1]), [0, P]],
                    )
                    in1 = bass.AP(
                        tensor=ypart.tensor,
                        offset=ypart.offset,
                        ap=[list(ypart.ap[0]), [0, P], list(ypart.ap[1])],
                    )
                    pt = work.tile([128, P, P], F32, tag="pair", name="pair_t")
                    tt_eng = nc.gpsimd if ob in gp_tt_obs else nc.vector
                    tt_eng.tensor_tensor(out=pt[:], in0=in0, in1=in1, op=OP.add)
                    st = work.tile([128, P, P], F32, tag="sig", name="sig_t")
                    nc.scalar.activation(out=st[:], in_=pt[:], func=AT.Sigmoid)
                    nc.vector.reduce_sum(
                        out=out_sb[ob][:, col:col + P],
                        in_=st[:],
                        axis=mybir.AxisListType.X,
                    )
                # scale by 1/denom (free-dim vector) on GpSimd, then store
                for ob in range(OB):
                    ot = outp.tile([128, P], F32, tag="ot", name="ot_t")
                    nc.gpsimd.tensor_tensor(
                        out=ot[:, :P],
                        in0=out_sb[ob][:, col:col + P],
                        in1=recip_rep[:, col:col + P],
                        op=OP.mult,
                    )
                    out_dma_engines[ob].dma_start(
                        out=out[ob * 128:(ob + 1) * 128, col:col + P],
                        in_=ot[:, :P],
                    )
                col += P

    nc.finalize()
    return nc


def kernel(num_graphs, nv, z, mask, Wc, bc, Wy, by):
    global _last_results
    G = int(num_graphs)
    NV = int(nv)
    z = np.ascontiguousarray(np.asarray(z, dtype=np.float32))
    mask = np.asarray(mask, dtype=np.float32).reshape(G, NV)
    Wc = np.asarray(Wc, dtype=np.float32)
    bc = np.asarray(bc, dtype=np.float32)
    Wy = np.asarray(Wy, dtype=np.float32)
    by = np.asarray(by, dtype=np.float32)
    H = z.shape[-1]
    zg = z.reshape(G, NV, H)

    out_full = np.zeros((G * NV, H), dtype=np.float32)

    # ---- host: active-node compaction & slot assignment ----
    act_idx = [np.nonzero(mask[g] > 0.5)[0] for g in range(G)]
    n_act = np.array([len(a) for a in act_idx])
    for g in range(G):
        if n_act[g] == 0:  # reference: 0/0 -> NaN for the whole graph
            out_full[g * NV:(g + 1) * NV, :] = np.nan

    order = np.argsort(-n_act, kind="stable")  # graphs by count, descending
    n_slots = (G + N_CORES - 1) // N_CORES
    assign = [[None] * n_slots for _ in range(N_CORES)]
    P_list = []
    for s in range(n_slots):
        ranks = order[s * N_CORES:(s + 1) * N_CORES]
        for c, g in enumerate(ranks):
            assign[c][s] = int(g)
        mx = max((int(n_act[g]) for g in ranks), default=0)
        P_list.append(max(2, mx))
    offs = np.cumsum([0] + P_list[:-1]).tolist()
    NTOT = sum(P_list)

    # ---- host: per-core input staging ----
    wcT = np.ascontiguousarray(Wc.T)  # [h_in, o]
    wyT = np.ascontiguousarray(Wy.T)
    OB = H // 128
    bcT = np.ascontiguousarray(bc.reshape(OB, 128).T)  # [128, OB]
    byT = np.ascontiguousarray(by.reshape(OB, 128).T)

    in_maps = []
    for c in range(N_CORES):
        zT_act = np.zeros((H, NTOT), dtype=np.float32)
        madd = np.full((1, NTOT), PAD_NEG, dtype=np.float32)
        recipN = np.zeros((1, NTOT), dtype=np.float32)
        for s in range(n_slots):
            g = assign[c][s]
            if g is None:
                continue
            n = int(n_act[g])
            if n == 0:
                continue
            o = int(offs[s])
            zT_act[:, o:o + n] = zg[g][act_idx[g]].T
            madd[0, o:o + n] = 0.0
            recipN[0, o:o + n] = np.float32(1.0) / np.float32(n)
        in_maps.append(
            {
                "zT": zT_act,
                "wcT": wcT,
                "wyT": wyT,
                "bcT": bcT,
                "byT": byT,
                "madd": madd,
                "recipN": recipN,
            }
        )

    # ---- build + run ----
    nc = _build_program(P_list, H)
    res = run_bass_kernel_spmd(nc, in_maps, list(range(N_CORES)))
    _last_results = res

    # ---- host: scatter back (untranspose [H, NTOT] -> rows) ----
    for c in range(N_CORES):
        oc = res.results[c]["out"]  # [H, NTOT]
        for s in range(n_slots):
            g = assign[c][s]
            if g is None:
                continue
            n = int(n_act[g])
            if n == 0:
                continue
            o = int(offs[s])
            out_full[g * NV + act_idx[g], :] = oc[:, o:o + n].T

    return out_full


# revision 9
# speedup vs baseline: 1.1950x; 1.1950x over previous
"""Trainium2 Bass kernel for masked pairwise-sigmoid GNN message passing.

Reference computation (per graph g with nodes i,j in [0,nv)):
    c = z @ Wc.T + bc ; y = z @ Wy.T + by          # [G, nv, H]
    s[g,i,j,:] = sigmoid(c[g,i,:] + y[g,j,:] + (m_i + m_j)*L - 2L)
    out[g,i,:] = sum_j s[g,i,j,:] / sum_j m[g,j]

Key exact identity: with m in {0,1}, any pair with m_i==0 or m_j==0 has
mask term <= -1e10, so sigmoid underflows to exactly 0 in fp32.  Hence
only "active" nodes (m==1) contribute, and for active pairs the mask
term is exactly 0.  The host gathers active nodes per graph, the device
computes the dense active x active interaction, and the host scatters
rows back (inactive rows are exactly 0).

Sharding: graphs are sorted by active count and dealt round-robin to the
8 cores in 4 "slots"; slot s is padded to a single global size P_s so
one SPMD program serves all cores.  Padding columns get a -1e5 additive
mask (sigmoid -> 0) and padding rows are discarded on scatter.

Device layout keeps the hidden dim on partitions throughout; the output
is returned transposed ([H, NTOT]) and the host untransposes during the
scatter (pure data movement).  All arithmetic including the 1/denom
scale happens on device.
"""

import numpy as np

import concourse.bass as bass
import concourse.mybir as mybir
import concourse.tile as tile
from concourse import bacc
from concourse.bass_utils import run_bass_kernel_spmd

F32 = mybir.dt.float32
F32R = mybir.dt.float32r
N_CORES = 8
PAD_NEG = -1.0e5  # additive mask for padding columns; sigmoid(-1e5) == 0

# test.py reads this for profiling info after a traced run
_last_results = None


def _build_program(P_list, H, gp_tt_obs=()):
    """One-core program; SPMD-replicated over 8 cores with different data.

    gp_tt_obs: which h-blocks' pairwise adds run on GpSimd instead of DVE
    (DVE is the bottleneck; GpSimd is otherwise idle).
    """
    NTOT = sum(P_list)
    KB = H // 128  # contraction blocks
    OB = H // 128  # output h blocks
    assert H % 128 == 0

    nc = bacc.Bacc(None, target_bir_lowering=False)

    zT = nc.dram_tensor("zT", [H, NTOT], F32, kind="ExternalInput")
    wcT = nc.dram_tensor("wcT", [H, H], F32, kind="ExternalInput")
    wyT = nc.dram_tensor("wyT", [H, H], F32, kind="ExternalInput")
    bcT = nc.dram_tensor("bcT", [128, OB], F32, kind="ExternalInput")
    byT = nc.dram_tensor("byT", [128, OB], F32, kind="ExternalInput")
    madd = nc.dram_tensor("madd", [1, NTOT], F32, kind="ExternalInput")
    recipN = nc.dram_tensor("recipN", [1, NTOT], F32, kind="ExternalInput")
    out = nc.dram_tensor("out", [H, NTOT], F32, kind="ExternalOutput")

    AT = mybir.ActivationFunctionType
    OP = mybir.AluOpType

    with tile.TileContext(nc) as tc:
        with (
            tc.tile_pool(name="singles", bufs=1) as singles,
            tc.tile_pool(name="work", bufs=3) as work,
            tc.tile_pool(name="outp", bufs=2) as outp,
            tc.tile_pool(name="psum", bufs=2, space="PSUM") as psum,
        ):
            # ---- load inputs; spread big loads over distinct engine queues
            z_sb = []
            for kb in range(KB):
                t = singles.tile([128, NTOT], F32, tag=f"z{kb}", name=f"z{kb}")
                nc.gpsimd.dma_start(out=t[:], in_=zT[kb * 128:(kb + 1) * 128, :])
                z_sb.append(t)
            w_sb = {}
            w_engines = {"c": nc.sync, "y": nc.scalar}
            for wname, dram in (("c", wcT), ("y", wyT)):
                for kb in range(KB):
                    t = singles.tile(
                        [128, H], F32, tag=f"w{wname}{kb}", name=f"w{wname}{kb}"
                    )
                    # split the [128, H] load into per-o-block halves so the
                    # first projection's weights land sooner
                    eng = w_engines[wname]
                    for ob in range(OB):
                        eng.dma_start(
                            out=t[:, ob * 128:(ob + 1) * 128],
                            in_=dram[kb * 128:(kb + 1) * 128,
                                     ob * 128:(ob + 1) * 128],
                        )
                    w_sb[wname, kb] = t
            bc_sb = singles.tile([128, OB], F32, tag="bc", name="bc_sb")
            nc.gpsimd.dma_start(out=bc_sb[:], in_=bcT[:])
            by_sb = singles.tile([128, OB], F32, tag="by", name="by_sb")
            nc.gpsimd.dma_start(out=by_sb[:], in_=byT[:])
            # one-row loads, broadcast to 128 partitions on-chip
            madd_row = singles.tile([1, NTOT], F32, tag="maddr", name="madd_row")
            nc.gpsimd.dma_start(out=madd_row[:], in_=madd[:])
            recip_row = singles.tile([1, NTOT], F32, tag="recr", name="recip_row")
            nc.gpsimd.dma_start(out=recip_row[:], in_=recipN[:])
            madd_rep = singles.tile([128, NTOT], F32, tag="madd", name="madd_rep")
            nc.gpsimd.partition_broadcast(madd_rep[:], madd_row[:])
            recip_rep = singles.tile([128, NTOT], F32, tag="recip", name="recip_rep")
            nc.gpsimd.partition_broadcast(recip_rep[:], recip_row[:])

            # ---- projections: C'[o, n] = (Wc @ z^T)[o, n] + bc[o] (+madd for y)
            ct_sb = [
                singles.tile([128, NTOT], F32, tag=f"ct{ob}", name=f"ct{ob}")
                for ob in range(OB)
            ]
            yt_sb = [
                singles.tile([128, NTOT], F32, tag=f"yt{ob}", name=f"yt{ob}")
                for ob in range(OB)
            ]
            # order (c,ob0), (y,ob0), (c,ob1), (y,ob1): the first slot's
            # pairwise add needs ob0 of both projections as early as possible
            proj_jobs = []
            for ob in range(OB):
                proj_jobs.append(("c", ob, ct_sb, bc_sb, False))
                proj_jobs.append(("y", ob, yt_sb, by_sb, True))
            for wname, ob, dest, bias_sb, with_madd in proj_jobs:
                ps = psum.tile([128, NTOT], F32, name="ps")
                for kb in range(KB):
                    nc.tensor.matmul(
                        ps[:],
                        lhsT=w_sb[wname, kb][:, ob * 128:(ob + 1) * 128],
                        rhs=z_sb[kb][:],
                        start=(kb == 0),
                        stop=(kb == KB - 1),
                    )
                if with_madd:
                    nc.vector.scalar_tensor_tensor(
                        out=dest[ob][:],
                        in0=ps[:],
                        scalar=bias_sb[:, ob:ob + 1],
                        in1=madd_rep[:],
                        op0=OP.add,
                        op1=OP.add,
                    )
                else:
                    nc.vector.tensor_scalar_add(
                        out=dest[ob][:], in0=ps[:], scalar1=bias_sb[:, ob:ob + 1]
                    )

            # ---- main loop; per-slot scale + store
            out_sb = [
                singles.tile([128, NTOT], F32, tag=f"o{ob}", name=f"osb{ob}")
                for ob in range(OB)
            ]
            out_dma_engines = [nc.sync, nc.sync]
            col = 0
            for si, P in enumerate(P_list):
                for ob in range(OB):
                    cpart = ct_sb[ob][:, col:col + P]  # [128, P] (i)
                    ypart = yt_sb[ob][:, col:col + P]  # [128, P] (j)
                    # in0[p, i, j] = c'[p, i]; in1[p, i, j] = y'[p, j]
                    in0 = bass.AP(
                        tensor=cpart.tensor,
                        offset=cpart.offset,
                        ap=[list(cpart.ap[0]), list(cpart.ap[1]), [0, P]],
                    )
                    in1 = bass.AP(
                        tensor=ypart.tensor,
                        offset=ypart.offset,
                        ap=[list(ypart.ap[0]), [0, P], list(ypart.ap[1])],
                    )
                    pt = work.tile([128, P, P], F32, tag="pair", name="pair_t")
                    tt_eng = nc.gpsimd if ob in gp_tt_obs else nc.vector
                    tt_eng.tensor_tensor(out=pt[:], in0=in0, in1=in1, op=OP.add)
                    st = work.tile([128, P, P], F32, tag="sig", name="sig_t")
                    nc.scalar.activation(out=st[:], in_=pt[:], func=AT.Sigmoid)
                    nc.vector.reduce_sum(
                        out=out_sb[ob][:, col:col + P],
                        in_=st[:],
                        axis=mybir.AxisListType.X,
                    )
                # scale by 1/denom (free-dim vector) on GpSimd, then store
                for ob in range(OB):
                    ot = outp.tile([128, P], F32, tag="ot", name="ot_t")
                    nc.vector.tensor_tensor(
                        out=ot[:, :P],
                        in0=out_sb[ob][:, col:col + P],
                        in1=recip_rep[:, col:col + P],
                        op=OP.mult,
                    )
                    out_dma_engines[ob].dma_start(
                        out=out[ob * 128:(ob + 1) * 128, col:col + P],
                        in_=ot[:, :P],
                    )
                col += P

    nc.finalize()
    return nc


def kernel(num_graphs, nv, z, mask, Wc, bc, Wy, by):
    global _last_results
    G = int(num_graphs)
    NV = int(nv)
    z = np.ascontiguousarray(np.asarray(z, dtype=np.float32))
    mask = np.asarray(mask, dtype=np.float32).reshape(G, NV)
    Wc = np.asarray(Wc, dtype=np.float32)
    bc = np.asarray(bc, dtype=np.float32)
    Wy = np.asarray(Wy, dtype=np.float32)
    by = np.asarray(by, dtype=np.float32)
    H = z.shape[-1]
    zg = z.reshape(G, NV, H)

    out_full = np.zeros((G * NV, H), dtype=np.float32)

    # ---- host: active-node compaction & slot assignment ----
    act_idx = [np.nonzero(mask[g] > 0.5)[0] for g in range(G)]
    n_act = np.array([len(a) for a in act_idx])
    for g in range(G):
        if n_act[g] == 0:  # reference: 0/0 -> NaN for the whole graph
            out_full[g * NV:(g + 1) * NV, :] = np.nan

    order = np.argsort(-n_act, kind="stable")  # graphs by count, descending
    n_slots = (G + N_CORES - 1) // N_CORES
    assign = [[None] * n_slots for _ in range(N_CORES)]
    P_list = []
    for s in range(n_slots):
        ranks = order[s * N_CORES:(s + 1) * N_CORES]
        for c, g in enumerate(ranks):
            assign[c][s] = int(g)
        mx = max((int(n_act[g]) for g in ranks), default=0)
        P_list.append(max(2, mx))
    offs = np.cumsum([0] + P_list[:-1]).tolist()
    NTOT = sum(P_list)

    # ---- host: per-core input staging ----
    wcT = np.ascontiguousarray(Wc.T)  # [h_in, o]
    wyT = np.ascontiguousarray(Wy.T)
    OB = H // 128
    bcT = np.ascontiguousarray(bc.reshape(OB, 128).T)  # [128, OB]
    byT = np.ascontiguousarray(by.reshape(OB, 128).T)

    in_maps = []
    for c in range(N_CORES):
        zT_act = np.zeros((H, NTOT), dtype=np.float32)
        madd = np.full((1, NTOT), PAD_NEG, dtype=np.float32)
        recipN = np.zeros((1, NTOT), dtype=np.float32)
        for s in range(n_slots):
            g = assign[c][s]
            if g is None:
                continue
            n = int(n_act[g])
            if n == 0:
                continue
            o = int(offs[s])
            zT_act[:, o:o + n] = zg[g][act_idx[g]].T
            madd[0, o:o + n] = 0.0
            recipN[0, o:o + n] = np.float32(1.0) / np.float32(n)
        in_maps.append(
            {
                "zT": zT_act,
                "wcT": wcT,
                "wyT": wyT,
                "bcT": bcT,
                "byT": byT,
                "madd": madd,
                "recipN": recipN,
            }
        )

    # ---- build + run ----
    nc = _build_program(P_list, H)
    res = run_bass_kernel_spmd(nc, in_maps, list(range(N_CORES)))
    _last_results = res

    # ---- host: scatter back (untranspose [H, NTOT] -> rows) ----
    for c in range(N_CORES):
        oc = res.results[c]["out"]  # [H, NTOT]
        for s in range(n_slots):
            g = assign[c][s]
            if g is None:
                continue
            n = int(n_act[g])
            if n == 0:
                continue
            o = int(offs[s])
            out_full[g * NV + act_idx[g], :] = oc[:, o:o + n].T

    return out_full


# revision 10
# speedup vs baseline: 1.2034x; 1.0071x over previous
"""Trainium2 Bass kernel for masked pairwise-sigmoid GNN message passing.

Reference computation (per graph g with nodes i,j in [0,nv)):
    c = z @ Wc.T + bc ; y = z @ Wy.T + by          # [G, nv, H]
    s[g,i,j,:] = sigmoid(c[g,i,:] + y[g,j,:] + (m_i + m_j)*L - 2L)
    out[g,i,:] = sum_j s[g,i,j,:] / sum_j m[g,j]

Key exact identity: with m in {0,1}, any pair with m_i==0 or m_j==0 has
mask term <= -1e10, so sigmoid underflows to exactly 0 in fp32.  Hence
only "active" nodes (m==1) contribute, and for active pairs the mask
term is exactly 0.  The host gathers active nodes per graph, the device
computes the dense active x active interaction, and the host scatters
rows back (inactive rows are exactly 0).

Sharding: graphs are sorted by active count and dealt round-robin to the
8 cores in 4 "slots"; slot s is padded to a single global size P_s so
one SPMD program serves all cores.  Padding columns get a -1e5 additive
mask (sigmoid -> 0) and padding rows are discarded on scatter.

Device layout keeps the hidden dim on partitions throughout; the output
is returned transposed ([H, NTOT]) and the host untransposes during the
scatter (pure data movement).  All arithmetic including the 1/denom
scale happens on device.
"""

import numpy as np

import concourse.bass as bass
import concourse.mybir as mybir
import concourse.tile as tile
from concourse import bacc
from concourse.bass_utils import run_bass_kernel_spmd

F32 = mybir.dt.float32
F32R = mybir.dt.float32r
N_CORES = 8
PAD_NEG = -1.0e5  # additive mask for padding columns; sigmoid(-1e5) == 0

# test.py reads this for profiling info after a traced run
_last_results = None


def _build_program(P_list, H, gp_tt_obs=()):
    """One-core program; SPMD-replicated over 8 cores with different data.

    gp_tt_obs: which h-blocks' pairwise adds run on GpSimd instead of DVE
    (DVE is the bottleneck; GpSimd is otherwise idle).
    """
    NTOT = sum(P_list)
    KB = H // 128  # contraction blocks
    OB = H // 128  # output h blocks
    assert H % 128 == 0

    nc = bacc.Bacc(None, target_bir_lowering=False)

    zT = nc.dram_tensor("zT", [H, NTOT], F32R, kind="ExternalInput")
    wcT = nc.dram_tensor("wcT", [H, H], F32R, kind="ExternalInput")
    wyT = nc.dram_tensor("wyT", [H, H], F32R, kind="ExternalInput")
    bcT = nc.dram_tensor("bcT", [128, OB], F32, kind="ExternalInput")
    byT = nc.dram_tensor("byT", [128, OB], F32, kind="ExternalInput")
    madd = nc.dram_tensor("madd", [128, NTOT], F32, kind="ExternalInput")
    recipN = nc.dram_tensor("recipN", [128, NTOT], F32, kind="ExternalInput")
    out = nc.dram_tensor("out", [H, NTOT], F32, kind="ExternalOutput")

    AT = mybir.ActivationFunctionType
    OP = mybir.AluOpType

    with tile.TileContext(nc) as tc:
        with (
            tc.tile_pool(name="singles", bufs=1) as singles,
            tc.tile_pool(name="work", bufs=3) as work,
            tc.tile_pool(name="outp", bufs=2) as outp,
            tc.tile_pool(name="psum", bufs=2, space="PSUM") as psum,
        ):
            # ---- load inputs; spread big loads over distinct engine queues
            z_sb = []
            for kb in range(KB):
                t = singles.tile([128, NTOT], F32R, tag=f"z{kb}", name=f"z{kb}")
                nc.gpsimd.dma_start(out=t[:], in_=zT[kb * 128:(kb + 1) * 128, :])
                z_sb.append(t)
            w_sb = {}
            w_engines = {"c": nc.sync, "y": nc.scalar}
            for wname, dram in (("c", wcT), ("y", wyT)):
                for kb in range(KB):
                    t = singles.tile(
                        [128, H], F32R, tag=f"w{wname}{kb}", name=f"w{wname}{kb}"
                    )
                    # split the [128, H] load into per-o-block halves so the
                    # first projection's weights land sooner
                    eng = w_engines[wname]
                    for ob in range(OB):
                        eng.dma_start(
                            out=t[:, ob * 128:(ob + 1) * 128],
                            in_=dram[kb * 128:(kb + 1) * 128,
                                     ob * 128:(ob + 1) * 128],
                        )
                    w_sb[wname, kb] = t
            bc_sb = singles.tile([128, OB], F32, tag="bc", name="bc_sb")
            nc.gpsimd.dma_start(out=bc_sb[:], in_=bcT[:])
            by_sb = singles.tile([128, OB], F32, tag="by", name="by_sb")
            nc.gpsimd.dma_start(out=by_sb[:], in_=byT[:])
            # host-replicated [128, NTOT] tensors: single contiguous DMAs
            madd_rep = singles.tile([128, NTOT], F32, tag="madd", name="madd_rep")
            nc.gpsimd.dma_start(out=madd_rep[:], in_=madd[:])
            recip_rep = singles.tile([128, NTOT], F32, tag="recip", name="recip_rep")
            nc.gpsimd.dma_start(out=recip_rep[:], in_=recipN[:])

            # ---- projections: C'[o, n] = (Wc @ z^T)[o, n] + bc[o] (+madd for y)
            ct_sb = [
                singles.tile([128, NTOT], F32, tag=f"ct{ob}", name=f"ct{ob}")
                for ob in range(OB)
            ]
            yt_sb = [
                singles.tile([128, NTOT], F32, tag=f"yt{ob}", name=f"yt{ob}")
                for ob in range(OB)
            ]
            # order (c,ob0), (y,ob0), (c,ob1), (y,ob1): the first slot's
            # pairwise add needs ob0 of both projections as early as possible
            proj_jobs = []
            for ob in range(OB):
                proj_jobs.append(("c", ob, ct_sb, bc_sb, False))
                proj_jobs.append(("y", ob, yt_sb, by_sb, True))
            for wname, ob, dest, bias_sb, with_madd in proj_jobs:
                ps = psum.tile([128, NTOT], F32, name="ps")
                for kb in range(KB):
                    nc.tensor.matmul(
                        ps[:],
                        lhsT=w_sb[wname, kb][:, ob * 128:(ob + 1) * 128],
                        rhs=z_sb[kb][:],
                        start=(kb == 0),
                        stop=(kb == KB - 1),
                    )
                if with_madd:
                    nc.vector.scalar_tensor_tensor(
                        out=dest[ob][:],
                        in0=ps[:],
                        scalar=bias_sb[:, ob:ob + 1],
                        in1=madd_rep[:],
                        op0=OP.add,
                        op1=OP.add,
                    )
                else:
                    nc.vector.tensor_scalar_add(
                        out=dest[ob][:], in0=ps[:], scalar1=bias_sb[:, ob:ob + 1]
                    )

            # ---- main loop; per-slot scale + store
            out_sb = [
                singles.tile([128, NTOT], F32, tag=f"o{ob}", name=f"osb{ob}")
                for ob in range(OB)
            ]
            out_dma_engines = [nc.sync, nc.sync]
            col = 0
            for si, P in enumerate(P_list):
                for ob in range(OB):
                    cpart = ct_sb[ob][:, col:col + P]  # [128, P] (i)
                    ypart = yt_sb[ob][:, col:col + P]  # [128, P] (j)
                    # in0[p, i, j] = c'[p, i]; in1[p, i, j] = y'[p, j]
                    in0 = bass.AP(
                        tensor=cpart.tensor,
                        offset=cpart.offset,
                        ap=[list(cpart.ap[0]), list(cpart.ap[1]), [0, P]],
                    )
                    in1 = bass.AP(
                        tensor=ypart.tensor,
                        offset=ypart.offset,
                        ap=[list(ypart.ap[0]), [0, P], list(ypart.ap[1])],
                    )
                    pt = work.tile([128, P, P], F32, tag="pair", name="pair_t")
                    tt_eng = nc.gpsimd if ob in gp_tt_obs else nc.vector
                    tt_eng.tensor_tensor(out=pt[:], in0=in0, in1=in1, op=OP.add)
                    st = work.tile([128, P, P], F32, tag="sig", name="sig_t")
                    nc.scalar.activation(out=st[:], in_=pt[:], func=AT.Sigmoid)
                    nc.vector.reduce_sum(
                        out=out_sb[ob][:, col:col + P],
                        in_=st[:],
                        axis=mybir.AxisListType.X,
                    )
                # scale by 1/denom (free-dim vector) on GpSimd, then store
                for ob in range(OB):
                    ot = outp.tile([128, P], F32, tag="ot", name="ot_t")
                    nc.gpsimd.tensor_tensor(
                        out=ot[:, :P],
                        in0=out_sb[ob][:, col:col + P],
                        in1=recip_rep[:, col:col + P],
                        op=OP.mult,
                    )
                    out_dma_engines[ob].dma_start(
                        out=out[ob * 128:(ob + 1) * 128, col:col + P],
                        in_=ot[:, :P],
                    )
                col += P

    nc.finalize()
    return nc


def kernel(num_graphs, nv, z, mask, Wc, bc, Wy, by):
    global _last_results
    G = int(num_graphs)
    NV = int(nv)
    z = np.ascontiguousarray(np.asarray(z, dtype=np.float32))
    mask = np.asarray(mask, dtype=np.float32).reshape(G, NV)
    Wc = np.asarray(Wc, dtype=np.float32)
    bc = np.asarray(bc, dtype=np.float32)
    Wy = np.asarray(Wy, dtype=np.float32)
    by = np.asarray(by, dtype=np.float32)
    H = z.shape[-1]
    zg = z.reshape(G, NV, H)

    out_full = np.zeros((G * NV, H), dtype=np.float32)

    # ---- host: active-node compaction & slot assignment ----
    act_idx = [np.nonzero(mask[g] > 0.5)[0] for g in range(G)]
    n_act = np.array([len(a) for a in act_idx])
    for g in range(G):
        if n_act[g] == 0:  # reference: 0/0 -> NaN for the whole graph
            out_full[g * NV:(g + 1) * NV, :] = np.nan

    order = np.argsort(-n_act, kind="stable")  # graphs by count, descending
    n_slots = (G + N_CORES - 1) // N_CORES
    assign = [[None] * n_slots for _ in range(N_CORES)]
    P_list = []
    for s in range(n_slots):
        ranks = order[s * N_CORES:(s + 1) * N_CORES]
        for c, g in enumerate(ranks):
            assign[c][s] = int(g)
        mx = max((int(n_act[g]) for g in ranks), default=0)
        P_list.append(max(2, mx))
    offs = np.cumsum([0] + P_list[:-1]).tolist()
    NTOT = sum(P_list)

    # ---- host: per-core input staging ----
    wcT = np.ascontiguousarray(Wc.T)  # [h_in, o]
    wyT = np.ascontiguousarray(Wy.T)
    OB = H // 128
    bcT = np.ascontiguousarray(bc.reshape(OB, 128).T)  # [128, OB]
    byT = np.ascontiguousarray(by.reshape(OB, 128).T)

    in_maps = []
    for c in range(N_CORES):
        zT_act = np.zeros((H, NTOT), dtype=np.float32)
        madd = np.full((1, NTOT), PAD_NEG, dtype=np.float32)
        recipN = np.zeros((1, NTOT), dtype=np.float32)
        for s in range(n_slots):
            g = assign[c][s]
            if g is None:
                continue
            n = int(n_act[g])
            if n == 0:
                continue
            o = int(offs[s])
            zT_act[:, o:o + n] = zg[g][act_idx[g]].T
            madd[0, o:o + n] = 0.0
            recipN[0, o:o + n] = np.float32(1.0) / np.float32(n)
        in_maps.append(
            {
                "zT": zT_act,
                "wcT": wcT,
                "wyT": wyT,
                "bcT": bcT,
                "byT": byT,
                "madd": np.ascontiguousarray(np.broadcast_to(madd, (128, NTOT))),
                "recipN": np.ascontiguousarray(np.broadcast_to(recipN, (128, NTOT))),
            }
        )

    # ---- build + run ----
    nc = _build_program(P_list, H)
    res = run_bass_kernel_spmd(nc, in_maps, list(range(N_CORES)))
    _last_results = res

    # ---- host: scatter back (untranspose [H, NTOT] -> rows) ----
    for c in range(N_CORES):
        oc = res.results[c]["out"]  # [H, NTOT]
        for s in range(n_slots):
            g = assign[c][s]
            if g is None:
                continue
            n = int(n_act[g])
            if n == 0:
                continue
            o = int(offs[s])
            out_full[g * NV + act_idx[g], :] = oc[:, o:o + n].T

    return out_full
